# revision 21
# baseline (speedup 1.0000x reference)
"""Trainium2 Bass kernel: causal multi-head attention block (B=2, S=2048, D=2048, H=16).

Sharding: tensor-parallel over heads. Each of the 8 cores owns 2 heads:
  - wq/wk/wv column-sharded (256 output features = 2 heads per core)
  - attention computed locally per head
  - AllGather of attention output (head axis), then wo row-sharded so each
    core produces a 256-wide column slice of the final output.

v2 structure: one fused pipeline over 8 token chunks of 512. Per chunk:
  QKV projection (merged 768-wide matmuls, x pre-transposed on host) ->
  RoPE + on-chip transposes -> attention for both heads -> chunked
  AllGather -> output projection (delayed one chunk to hide the AG).
All matmuls bf16 with fp32 accumulation; scores computed transposed
(sT[j,i]) so exp(sT) feeds the PV matmul directly; row sums via a
ones-matmul; causal partial tiles trimmed to their valid column range.
"""

import sys

sys.path.insert(0, "/opt/trn_rl_repo")

import numpy as np
import ml_dtypes

B, S, D, H = 2, 2048, 2048, 16
HD = 128          # head dim
NCORES = 8
HPC = H // NCORES  # heads per core = 2
ESH = HPC * HD     # per-core feature shard = 256
T = B * S          # flattened tokens = 4096
NT = T // 128      # token tiles = 32
ND = D // 128      # feature tiles = 16
NCHUNK = T // 512  # pipeline chunks = 8
SCALE = 1.0 / np.sqrt(HD)

_cache = {}


def _build_nc():
    import concourse.bass as bass
    import concourse.mybir as mybir
    import concourse.tile as tile
    from concourse import bacc
    from concourse.masks import make_identity

    f32 = mybir.dt.float32
    bf16 = mybir.dt.bfloat16

    nc = bacc.Bacc(None, target_bir_lowering=False, num_devices=NCORES)

    # ---- kernel I/O (per-core shards, prepared on host) ----
    xTd = nc.declare_dram_parameter("xT", [D, T], bf16, isOutput=False)
    wcatT = nc.declare_dram_parameter("wcatT", [D, 3 * ESH], bf16, isOutput=False)
    woT = nc.declare_dram_parameter("woT", [D, ESH], bf16, isOutput=False)
    cosf = nc.declare_dram_parameter("cosf", [S, HD], bf16, isOutput=False)
    sinf = nc.declare_dram_parameter("sinf", [S, HD], bf16, isOutput=False)
    maskdT = nc.declare_dram_parameter("maskdT", [S, 128], f32, isOutput=False)
    yT = nc.declare_dram_parameter("out", [ESH, T], f32, isOutput=True)

    # collective bounce buffers (internal DRAM), one pair per token chunk;
    # the last chunk is split in two halves so its AllGather pipelines with
    # the final projection instead of sitting fully exposed at the tail
    agin = [nc.dram_tensor(f"agin{i}", [ESH, 512], bf16)
            for i in range(NCHUNK - 1)]
    agout = [nc.dram_tensor(f"agout{i}", [D, 512], bf16, addr_space="Shared")
             for i in range(NCHUNK - 1)]
    agin_h = [nc.dram_tensor(f"aginh{i}", [ESH, 256], bf16) for i in range(2)]
    agout_h = [nc.dram_tensor(f"agouth{i}", [D, 256], bf16, addr_space="Shared")
               for i in range(2)]

    with tile.TileContext(nc) as tc:
        with (
            tc.tile_pool(name="const", bufs=1) as constp,
            tc.tile_pool(name="wpool", bufs=1) as wpool,
            tc.tile_pool(name="qkv", bufs=1) as qkvp,
            tc.tile_pool(name="xt", bufs=20) as xtp,
            tc.tile_pool(name="qkr", bufs=3) as qkrp,
            tc.tile_pool(name="rope", bufs=6) as ropep,
            tc.tile_pool(name="pt", bufs=20) as ptp,
            tc.tile_pool(name="sm", bufs=4) as smp,
            tc.tile_pool(name="ot", bufs=4) as otp,
            tc.tile_pool(name="ys", bufs=3) as ysp,
            tc.tile_pool(name="yrhs", bufs=18) as yrhsp,
            tc.tile_pool(name="psQ", bufs=1, space="PSUM") as psQ,
            tc.tile_pool(name="psQb", bufs=1, space="PSUM") as psQb,
            tc.tile_pool(name="psS", bufs=3, space="PSUM") as psS,
            tc.tile_pool(name="psC", bufs=1, space="PSUM") as psC,
            tc.tile_pool(name="psB", bufs=2, space="PSUM") as psB,
        ):
            # ---- constants ----
            ident = constp.tile([128, 128], bf16, tag="ident")
            make_identity(nc, ident[:])
            f32r = mybir.dt.float32r
            ones_f = constp.tile([128, 128], f32, tag="ones_f")
            nc.vector.memset(ones_f[:], 1.0)
            ones = constp.tile([128, 128], f32r, tag="ones")
            nc.vector.tensor_scalar_mul(ones[:], ones_f[:], 1.0)

            # resident weights: wcatT tiles [128d, 768]
            wcat_sb = []
            for dt in range(ND):
                t_ = wpool.tile([128, 3 * ESH], bf16, tag=f"wc{dt}")
                nc.sync.dma_start(t_[:], wcatT[dt * 128:(dt + 1) * 128, :])
                wcat_sb.append(t_)
            # rope tables, natural layout [t-part, hd-free] per s-tile
            cos_sb = constp.tile([128, S // 128 * HD], bf16, tag="cos")
            sin_sb = constp.tile([128, S // 128 * HD], bf16, tag="sin")
            nc.sync.dma_start(
                cos_sb[:].rearrange("p (n d) -> p n d", d=HD),
                cosf.rearrange("(n p) d -> p n d", p=128))
            nc.sync.dma_start(
                sin_sb[:].rearrange("p (n d) -> p n d", d=HD),
                sinf.rearrange("(n p) d -> p n d", p=128))

            # transposed diagonal mask blocks [j-part, i-free] per s-tile
            maskd_sb = constp.tile([128, S], f32, tag="maskd")
            nc.scalar.dma_start(
                maskd_sb[:].rearrange("p (n d) -> p n d", d=128),
                maskdT.rearrange("(n p) d -> p n d", p=128))

            # wo tiles off the critical startup path (scalar queue)
            wo_sb = []
            for dt in range(ND):
                t_ = wpool.tile([128, ESH], bf16, tag=f"wo{dt}")
                nc.scalar.dma_start(t_[:], woT[dt * 128:(dt + 1) * 128, :])
                wo_sb.append(t_)

            # persistent attention operands (bf16)
            # qT/kT: [(h,b)] -> [128hd, S]  (head-major, feature on partitions)
            # v    : [(h,b)] -> [128s-in-tile, S/128 * HD] natural blocks
            qT = {}
            kT = {}
            vN = {}
            for h in range(HPC):
                for b in range(B):
                    qT[h, b] = qkvp.tile([128, S], bf16, tag=f"q{h}{b}",
                                         name=f"qT{h}{b}")
                    kT[h, b] = qkvp.tile([128, S], bf16, tag=f"k{h}{b}",
                                         name=f"kT{h}{b}")
                    vN[h, b] = qkvp.tile([128, S // 128 * HD], bf16,
                                         tag=f"v{h}{b}", name=f"vN{h}{b}")

            # deferred PE transpose ops; popped between later matmuls so the
            # psC bank drain hides behind matmul execution
            pending = []

            def pop_pending(n=1):
                for _ in range(min(n, len(pending))):
                    pending.pop(0)()

            def flush_pending():
                pop_pending(len(pending))

            # ---------------- stage: QKV + RoPE for one chunk ----------------
            def qkv_chunk(tc_i):
                xt_tiles = []
                for dt in range(ND):
                    xt = xtp.tile([128, 512], bf16, tag="xt")
                    nc.sync.dma_start(
                        xt[:],
                        xTd[dt * 128:(dt + 1) * 128,
                            tc_i * 512:(tc_i + 1) * 512])
                    xt_tiles.append(xt)
                for tt in range(4):               # token tiles within chunk
                    gt = tc_i * 4 + tt            # global token tile 0..31
                    b = gt // (S // 128)
                    st = gt % (S // 128)          # s-tile within batch
                    pa = psQ.tile([128, 512], f32, tag="pa")
                    pb = psQb.tile([128, 256], f32, tag="pb")
                    for dt in range(ND):
                        lhsT = xt_tiles[dt][:, tt * 128:(tt + 1) * 128]
                        # pb first: its matmul covers the tail of pa's drain
                        nc.tensor.matmul(pb[:], lhsT, wcat_sb[dt][:, 512:768],
                                         start=(dt == 0), stop=(dt == ND - 1))
                        nc.tensor.matmul(pa[:], lhsT, wcat_sb[dt][:, 0:512],
                                         start=(dt == 0), stop=(dt == ND - 1))
                        # hide previous token tile's transposes behind the
                        # accumulation matmuls (they only need the PE briefly)
                        if dt >= 8 and dt % 2 == 0:
                            pop_pending()
                    # drain psum: q|k -> qkraw (vector), v -> natural (scalar)
                    qkraw = qkrp.tile([128, 512], bf16, tag="qkraw")
                    nc.vector.tensor_scalar_mul(qkraw[:], pa[:], 1.0)
                    for h in range(HPC):
                        nc.scalar.activation(
                            vN[h, b][:, st * HD:(st + 1) * HD],
                            pb[:, h * HD:(h + 1) * HD],
                            mybir.ActivationFunctionType.Copy)
                    # q/k rope then transpose
                    cos_t = cos_sb[:, st * HD:(st + 1) * HD]
                    sin_t = sin_sb[:, st * HD:(st + 1) * HD]
                    for which, base in (("q", 0), ("k", 256)):
                        for h in range(HPC):
                            qn = qkraw[:, base + h * HD:base + (h + 1) * HD]
                            acc = ropep.tile([128, HD], bf16, tag="acc")
                            swp = ropep.tile([128, HD], bf16, tag="swp")
                            # acc = qn * cos ; swp = pairswap(qn) * sin_signed
                            nc.vector.tensor_mul(acc[:], qn, cos_t)
                            nc.vector.tensor_mul(
                                swp[:, 0:HD:2], qn[:, 1:HD:2], sin_t[:, 0:HD:2])
                            nc.vector.tensor_mul(
                                swp[:, 1:HD:2], qn[:, 0:HD:2], sin_t[:, 1:HD:2])
                            rot = ropep.tile([128, HD], bf16, tag="rot")
                            nc.vector.tensor_add(rot[:], acc[:], swp[:])
                            dst = qT[h, b] if which == "q" else kT[h, b]

                            def do_transpose(rot=rot, dst=dst, st=st):
                                pc = psC.tile([128, 128], bf16, tag="C")
                                nc.tensor.transpose(pc[:], rot[:], ident[:])
                                nc.scalar.activation(
                                    dst[:, st * 128:(st + 1) * 128], pc[:],
                                    mybir.ActivationFunctionType.Copy)

                            pending.append(do_transpose)

            # ---------------- attention phases ----------------
            def scores_phase(h, b, c):
                J = 4 * (c + 1)                   # causal j-tiles
                # partial row sums accumulated across j-tiles on the (idle)
                # gpsimd engine; one ones-matmul per head-chunk then reduces
                # the partition axis instead of J of them
                pacc = smp.tile([128, 512], f32r, tag="pacc")
                pts = []
                for jt in range(J):
                    # i-range (local to chunk) this j-tile can see
                    lo = max(0, jt * 128 - c * 512)
                    ps = psS.tile([128, 512], f32, tag="S")
                    nc.tensor.matmul(
                        ps[:, lo:512],
                        kT[h, b][:, jt * 128:(jt + 1) * 128],
                        qT[h, b][:, c * 512 + lo:(c + 1) * 512],
                        start=True, stop=True)
                    pop_pending()
                    if jt >= 4 * c:
                        # diagonal block: add transposed mask
                        dl = jt * 128 - c * 512
                        nc.vector.tensor_add(
                            ps[:, dl:dl + 128], ps[:, dl:dl + 128],
                            maskd_sb[:, jt * 128:(jt + 1) * 128])
                    pt = ptp.tile([128, 512], bf16, tag="pt")
                    nc.scalar.activation(
                        pt[:, lo:512], ps[:, lo:512],
                        mybir.ActivationFunctionType.Exp, scale=SCALE)
                    if jt == 0:
                        nc.gpsimd.tensor_scalar_mul(pacc[:], pt[:], 1.0)
                    else:
                        nc.gpsimd.tensor_add(
                            pacc[:, lo:512], pacc[:, lo:512], pt[:, lo:512])
                    pts.append((pt, lo))
                return pts, pacc

            def finish_phase(h, b, c, pts, pacc):
                J = 4 * (c + 1)
                # row sums: single f32r matmul contracting the partition axis
                pr = psB.tile([128, 512], f32, tag="B")
                nc.tensor.matmul(pr[:], ones[:], pacc[:],
                                 start=True, stop=True)
                # PV
                po = psB.tile([128, 512], f32, tag="B")
                for jt in range(J):
                    pt, lo = pts[jt]
                    nc.tensor.matmul(
                        po[:, lo:512], vN[h, b][:, jt * HD:(jt + 1) * HD],
                        pt[:, lo:512], start=(jt == 0), stop=(jt == J - 1),
                        skip_group_check=True)
                rbc = smp.tile([128, 512], f32, tag="rbc")
                nc.vector.reciprocal_approx_fast(rbc[:], pr[:])
                ot = otp.tile([128, 512], bf16, tag="ot")
                nc.vector.tensor_mul(ot[:], po[:], rbc[:])
                ci = b * 4 + c
                if ci < NCHUNK - 1:
                    nc.scalar.dma_start(
                        agin[ci][h * 128:(h + 1) * 128, :], ot[:])
                else:
                    for half in range(2):
                        nc.scalar.dma_start(
                            agin_h[half][h * 128:(h + 1) * 128, :],
                            ot[:, half * 256:(half + 1) * 256])

            # ---------------- output projection ----------------
            def rhs_load(src, w):
                rhs_tiles = []
                for et in range(ND):
                    rt = yrhsp.tile([128, 512], bf16, tag="yr")
                    nc.sync.dma_start(
                        rt[:, 0:w], src[et * 128:(et + 1) * 128, :])
                    rhs_tiles.append(rt)
                return rhs_tiles

            def project_mms(k, rhs_tiles, i0=0, w=512):
                for ft in range(ESH // 128):
                    py = psB.tile([128, 512], f32, tag="B")
                    for et in range(ND):
                        nc.tensor.matmul(
                            py[:, 0:w], wo_sb[et][:, ft * 128:(ft + 1) * 128],
                            rhs_tiles[et][:, 0:w],
                            start=(et == 0), stop=(et == ND - 1))
                    ys = ysp.tile([128, 512], f32, tag="ys")
                    nc.scalar.activation(ys[:, 0:w], py[:, 0:w],
                                         mybir.ActivationFunctionType.Copy)
                    nc.scalar.dma_start(
                        yT[ft * 128:(ft + 1) * 128,
                           k * 512 + i0:k * 512 + i0 + w],
                        ys[:, 0:w])

            def all_gather(in_ap, out_ap):
                nc.gpsimd.collective_compute(
                    "AllGather",
                    mybir.AluOpType.bypass,
                    replica_groups=[list(range(NCORES))],
                    ins=[in_ap],
                    outs=[out_ap],
                )

            # ---------------- fused pipeline ----------------
            prev_rhs = None
            for ci in range(NCHUNK):
                b, c = divmod(ci, S // 512)
                if ci >= 1:
                    prev_rhs = rhs_load(agout[ci - 1], 512)
                qkv_chunk(ci)
                flush_pending()
                pts0, pacc0 = scores_phase(0, b, c)
                if ci >= 1:
                    project_mms(ci - 1, prev_rhs)
                finish_phase(0, b, c, pts0, pacc0)
                pts1, pacc1 = scores_phase(1, b, c)
                finish_phase(1, b, c, pts1, pacc1)
                if ci < NCHUNK - 1:
                    all_gather(agin[ci].ap().opt(), agout[ci].ap().opt())
                else:
                    # last chunk: two half-width AllGathers so the first
                    # half's projection overlaps the second half's gather
                    for half in range(2):
                        all_gather(agin_h[half].ap().opt(),
                                   agout_h[half].ap().opt())
            for half in range(2):
                rhs = rhs_load(agout_h[half], 256)
                project_mms(NCHUNK - 1, rhs, i0=half * 256, w=256)
    nc.finalize()
    return nc


def _prep_inputs(x, wq, wk, wv, wo, freqs_cos, freqs_sin, mask):
    bf16 = ml_dtypes.bfloat16
    xf = np.ascontiguousarray(x.reshape(T, D).T).astype(bf16)
    # rope tables expanded to head-dim width; sin carries the rotation signs
    cosf = np.repeat(freqs_cos, 2, axis=1).astype(bf16)          # [S, 128]
    sinf = np.repeat(freqs_sin, 2, axis=1)
    sinf = (sinf * np.tile([-1.0, 1.0], HD // 2)[None, :]).astype(bf16)
    # transposed diagonal mask blocks, stacked: [S, 128]
    mdT = np.concatenate(
        [np.ascontiguousarray(mask[i * 128:(i + 1) * 128,
                                   i * 128:(i + 1) * 128].T)
         for i in range(S // 128)], axis=0).astype(np.float32)
    in_maps = []
    for c in range(NCORES):
        sl = slice(c * ESH, (c + 1) * ESH)
        wcatT = np.concatenate(
            [wq[sl, :].T, wk[sl, :].T, wv[sl, :].T], axis=1).astype(bf16)
        woTc = np.ascontiguousarray(wo[sl, :].T).astype(bf16)
        in_maps.append({
            "xT": xf,
            "wcatT": np.ascontiguousarray(wcatT),
            "woT": woTc,
            "cosf": cosf,
            "sinf": sinf,
            "maskdT": mdT,
        })
    return in_maps


def kernel(x, wq, wk, wv, wo, freqs_cos, freqs_sin, mask, start_pos):
    from concourse.bass_utils import run_bass_kernel_spmd

    x = np.asarray(x, dtype=np.float32)
    in_maps = _prep_inputs(
        np.asarray(x, np.float32), np.asarray(wq, np.float32),
        np.asarray(wk, np.float32), np.asarray(wv, np.float32),
        np.asarray(wo, np.float32), np.asarray(freqs_cos, np.float32),
        np.asarray(freqs_sin, np.float32), np.asarray(mask, np.float32))

    if "nc" not in _cache:
        _cache["nc"] = _build_nc()
    res = run_bass_kernel_spmd(_cache["nc"], in_maps, core_ids=list(range(NCORES)))
    _cache["last_result"] = res

    y = np.empty((T, D), dtype=np.float32)
    for c in range(NCORES):
        y[:, c * ESH:(c + 1) * ESH] = np.asarray(res.results[c]["out"]).T
    return y.reshape(B, S, D)


# revision 22
# speedup vs baseline: 1.1779x; 1.1779x over previous
"""Trainium2 Bass kernel: causal multi-head attention block (B=2, S=2048, D=2048, H=16).

Sharding: tensor-parallel over heads. Each of the 8 cores owns 2 heads:
  - wq/wk/wv column-sharded (256 output features = 2 heads per core)
  - attention computed locally per head
  - AllGather of attention output (head axis), then wo row-sharded so each
    core produces a 256-wide column slice of the final output.

v2 structure: one fused pipeline over 8 token chunks of 512. Per chunk:
  QKV projection (merged 768-wide matmuls, x pre-transposed on host) ->
  RoPE + on-chip transposes -> attention for both heads -> chunked
  AllGather -> output projection (delayed one chunk to hide the AG).
All matmuls bf16 with fp32 accumulation; scores computed transposed
(sT[j,i]) so exp(sT) feeds the PV matmul directly; row sums via a
ones-matmul; causal partial tiles trimmed to their valid column range.
"""

import sys

sys.path.insert(0, "/opt/trn_rl_repo")

import numpy as np
import ml_dtypes

B, S, D, H = 2, 2048, 2048, 16
HD = 128          # head dim
NCORES = 8
HPC = H // NCORES  # heads per core = 2
ESH = HPC * HD     # per-core feature shard = 256
T = B * S          # flattened tokens = 4096
NT = T // 128      # token tiles = 32
ND = D // 128      # feature tiles = 16
NCHUNK = T // 512  # pipeline chunks = 8
SCALE = 1.0 / np.sqrt(HD)

_cache = {}


def _build_nc():
    import concourse.bass as bass
    import concourse.mybir as mybir
    import concourse.tile as tile
    from concourse import bacc
    from concourse.masks import make_identity

    f32 = mybir.dt.float32
    bf16 = mybir.dt.bfloat16

    nc = bacc.Bacc(None, target_bir_lowering=False, num_devices=NCORES)

    # ---- kernel I/O (per-core shards, prepared on host) ----
    xTd = nc.declare_dram_parameter("xT", [D, T], bf16, isOutput=False)
    wcatT = nc.declare_dram_parameter("wcatT", [D, 3 * ESH], bf16, isOutput=False)
    woT = nc.declare_dram_parameter("woT", [D, ESH], bf16, isOutput=False)
    cosf = nc.declare_dram_parameter("cosf", [S, HD], bf16, isOutput=False)
    sinf = nc.declare_dram_parameter("sinf", [S, HD], bf16, isOutput=False)
    maskdT = nc.declare_dram_parameter("maskdT", [S, 128], f32, isOutput=False)
    yT = nc.declare_dram_parameter("out", [ESH, T], f32, isOutput=True)

    # collective bounce buffers (internal DRAM), one pair per token chunk;
    # the last chunk is split in two halves so its AllGather pipelines with
    # the final projection instead of sitting fully exposed at the tail
    agin = [nc.dram_tensor(f"agin{i}", [ESH, 512], bf16)
            for i in range(NCHUNK - 1)]
    agout = [nc.dram_tensor(f"agout{i}", [D, 512], bf16, addr_space="Shared")
             for i in range(NCHUNK - 1)]
    agin_h = [nc.dram_tensor(f"aginh{i}", [ESH, 256], bf16) for i in range(2)]
    agout_h = [nc.dram_tensor(f"agouth{i}", [D, 256], bf16, addr_space="Shared")
               for i in range(2)]

    with tile.TileContext(nc) as tc:
        with (
            tc.tile_pool(name="const", bufs=1) as constp,
            tc.tile_pool(name="wpool", bufs=1) as wpool,
            tc.tile_pool(name="qkv", bufs=1) as qkvp,
            tc.tile_pool(name="xt", bufs=20) as xtp,
            tc.tile_pool(name="qkr", bufs=3) as qkrp,
            tc.tile_pool(name="rope", bufs=6) as ropep,
            tc.tile_pool(name="pt", bufs=20) as ptp,
            tc.tile_pool(name="sm", bufs=4) as smp,
            tc.tile_pool(name="ot", bufs=4) as otp,
            tc.tile_pool(name="ys", bufs=3) as ysp,
            tc.tile_pool(name="yrhs", bufs=18) as yrhsp,
            tc.tile_pool(name="psQ", bufs=1, space="PSUM") as psQ,
            tc.tile_pool(name="psQb", bufs=1, space="PSUM") as psQb,
            tc.tile_pool(name="psS", bufs=3, space="PSUM") as psS,
            tc.tile_pool(name="psC", bufs=1, space="PSUM") as psC,
            tc.tile_pool(name="psB", bufs=2, space="PSUM") as psB,
        ):
            # ---- constants ----
            ident = constp.tile([128, 128], bf16, tag="ident")
            make_identity(nc, ident[:])
            f32r = mybir.dt.float32r
            ones_f = constp.tile([128, 128], f32, tag="ones_f")
            nc.vector.memset(ones_f[:], 1.0)
            ones = constp.tile([128, 128], f32r, tag="ones")
            nc.vector.tensor_scalar_mul(ones[:], ones_f[:], 1.0)

            # resident weights: wcatT tiles [128d, 768]
            wcat_sb = []
            for dt in range(ND):
                t_ = wpool.tile([128, 3 * ESH], bf16, tag=f"wc{dt}")
                nc.sync.dma_start(t_[:], wcatT[dt * 128:(dt + 1) * 128, :])
                wcat_sb.append(t_)
            # rope tables, natural layout [t-part, hd-free] per s-tile
            cos_sb = constp.tile([128, S // 128 * HD], bf16, tag="cos")
            sin_sb = constp.tile([128, S // 128 * HD], bf16, tag="sin")
            nc.sync.dma_start(
                cos_sb[:].rearrange("p (n d) -> p n d", d=HD),
                cosf.rearrange("(n p) d -> p n d", p=128))
            nc.sync.dma_start(
                sin_sb[:].rearrange("p (n d) -> p n d", d=HD),
                sinf.rearrange("(n p) d -> p n d", p=128))

            # transposed diagonal mask blocks [j-part, i-free] per s-tile
            maskd_sb = constp.tile([128, S], f32, tag="maskd")
            nc.scalar.dma_start(
                maskd_sb[:].rearrange("p (n d) -> p n d", d=128),
                maskdT.rearrange("(n p) d -> p n d", p=128))

            # wo tiles off the critical startup path (scalar queue)
            wo_sb = []
            for dt in range(ND):
                t_ = wpool.tile([128, ESH], bf16, tag=f"wo{dt}")
                nc.scalar.dma_start(t_[:], woT[dt * 128:(dt + 1) * 128, :])
                wo_sb.append(t_)

            # persistent attention operands (bf16)
            # qT/kT: [(h,b)] -> [128hd, S]  (head-major, feature on partitions)
            # v    : [(h,b)] -> [128s-in-tile, S/128 * HD] natural blocks
            qT = {}
            kT = {}
            vN = {}
            for h in range(HPC):
                for b in range(B):
                    qT[h, b] = qkvp.tile([128, S], bf16, tag=f"q{h}{b}",
                                         name=f"qT{h}{b}")
                    kT[h, b] = qkvp.tile([128, S], bf16, tag=f"k{h}{b}",
                                         name=f"kT{h}{b}")
                    vN[h, b] = qkvp.tile([128, S // 128 * HD], bf16,
                                         tag=f"v{h}{b}", name=f"vN{h}{b}")

            # deferred PE transpose ops; popped between later matmuls so the
            # psC bank drain hides behind matmul execution
            pending = []

            def pop_pending(n=1):
                for _ in range(min(n, len(pending))):
                    pending.pop(0)()

            def flush_pending():
                pop_pending(len(pending))

            # ---------------- stage: QKV + RoPE for one chunk ----------------
            def qkv_chunk(tc_i):
                xt_tiles = []
                for dt in range(ND):
                    xt = xtp.tile([128, 512], bf16, tag="xt")
                    nc.sync.dma_start(
                        xt[:],
                        xTd[dt * 128:(dt + 1) * 128,
                            tc_i * 512:(tc_i + 1) * 512])
                    xt_tiles.append(xt)
                for tt in range(4):               # token tiles within chunk
                    gt = tc_i * 4 + tt            # global token tile 0..31
                    b = gt // (S // 128)
                    st = gt % (S // 128)          # s-tile within batch
                    pa = psQ.tile([128, 512], f32, tag="pa")
                    pb = psQb.tile([128, 256], f32, tag="pb")
                    for dt in range(ND):
                        lhsT = xt_tiles[dt][:, tt * 128:(tt + 1) * 128]
                        # pb first: its matmul covers the tail of pa's drain
                        nc.tensor.matmul(pb[:], lhsT, wcat_sb[dt][:, 512:768],
                                         start=(dt == 0), stop=(dt == ND - 1))
                        nc.tensor.matmul(pa[:], lhsT, wcat_sb[dt][:, 0:512],
                                         start=(dt == 0), stop=(dt == ND - 1))
                        # hide previous token tile's transposes behind the
                        # accumulation matmuls (they only need the PE briefly)
                        if dt >= 8 and dt % 2 == 0:
                            pop_pending()
                    # drain psum: q|k -> qkraw (vector), v -> natural (scalar)
                    qkraw = qkrp.tile([128, 512], bf16, tag="qkraw")
                    nc.vector.tensor_scalar_mul(qkraw[:], pa[:], 1.0)
                    for h in range(HPC):
                        nc.scalar.activation(
                            vN[h, b][:, st * HD:(st + 1) * HD],
                            pb[:, h * HD:(h + 1) * HD],
                            mybir.ActivationFunctionType.Copy)
                    # q/k rope then transpose
                    cos_t = cos_sb[:, st * HD:(st + 1) * HD]
                    sin_t = sin_sb[:, st * HD:(st + 1) * HD]
                    for which, base in (("q", 0), ("k", 256)):
                        for h in range(HPC):
                            qn = qkraw[:, base + h * HD:base + (h + 1) * HD]
                            acc = ropep.tile([128, HD], bf16, tag="acc")
                            swp = ropep.tile([128, HD], bf16, tag="swp")
                            # acc = qn * cos ; swp = pairswap(qn) * sin_signed
                            nc.vector.tensor_mul(acc[:], qn, cos_t)
                            nc.vector.tensor_mul(
                                swp[:, 0:HD:2], qn[:, 1:HD:2], sin_t[:, 0:HD:2])
                            nc.vector.tensor_mul(
                                swp[:, 1:HD:2], qn[:, 0:HD:2], sin_t[:, 1:HD:2])
                            rot = ropep.tile([128, HD], bf16, tag="rot")
                            nc.vector.tensor_add(rot[:], acc[:], swp[:])
                            dst = qT[h, b] if which == "q" else kT[h, b]

                            def do_transpose(rot=rot, dst=dst, st=st):
                                pc = psC.tile([128, 128], bf16, tag="C")
                                nc.tensor.transpose(pc[:], rot[:], ident[:])
                                nc.scalar.activation(
                                    dst[:, st * 128:(st + 1) * 128], pc[:],
                                    mybir.ActivationFunctionType.Copy)

                            pending.append(do_transpose)

            # ---------------- attention phases ----------------
            def scores_phase(h, b, c):
                J = 4 * (c + 1)                   # causal j-tiles
                # partial row sums accumulated across j-tiles on the (idle)
                # gpsimd engine; one ones-matmul per head-chunk then reduces
                # the partition axis instead of J of them
                pacc = smp.tile([128, 512], f32r, tag="pacc")
                pts = []
                for jt in range(J):
                    # i-range (local to chunk) this j-tile can see
                    lo = max(0, jt * 128 - c * 512)
                    ps = psS.tile([128, 512], f32, tag="S")
                    nc.tensor.matmul(
                        ps[:, lo:512],
                        kT[h, b][:, jt * 128:(jt + 1) * 128],
                        qT[h, b][:, c * 512 + lo:(c + 1) * 512],
                        start=True, stop=True)
                    pop_pending()
                    if jt >= 4 * c:
                        # diagonal block: add transposed mask
                        dl = jt * 128 - c * 512
                        nc.vector.tensor_add(
                            ps[:, dl:dl + 128], ps[:, dl:dl + 128],
                            maskd_sb[:, jt * 128:(jt + 1) * 128])
                    pt = ptp.tile([128, 512], bf16, tag="pt")
                    nc.scalar.activation(
                        pt[:, lo:512], ps[:, lo:512],
                        mybir.ActivationFunctionType.Exp, scale=SCALE)
                    if jt == 0:
                        nc.vector.tensor_scalar_mul(pacc[:], pt[:], 1.0)
                    else:
                        nc.vector.tensor_add(
                            pacc[:, lo:512], pacc[:, lo:512], pt[:, lo:512])
                    pts.append((pt, lo))
                return pts, pacc

            def finish_phase(h, b, c, pts, pacc):
                J = 4 * (c + 1)
                # row sums: single f32r matmul contracting the partition axis
                pr = psB.tile([128, 512], f32, tag="B")
                nc.tensor.matmul(pr[:], ones[:], pacc[:],
                                 start=True, stop=True)
                # PV
                po = psB.tile([128, 512], f32, tag="B")
                for jt in range(J):
                    pt, lo = pts[jt]
                    nc.tensor.matmul(
                        po[:, lo:512], vN[h, b][:, jt * HD:(jt + 1) * HD],
                        pt[:, lo:512], start=(jt == 0), stop=(jt == J - 1),
                        skip_group_check=True)
                rbc = smp.tile([128, 512], f32, tag="rbc")
                nc.vector.reciprocal_approx_fast(rbc[:], pr[:])
                ot = otp.tile([128, 512], bf16, tag="ot")
                nc.vector.tensor_mul(ot[:], po[:], rbc[:])
                ci = b * 4 + c
                if ci < NCHUNK - 1:
                    nc.scalar.dma_start(
                        agin[ci][h * 128:(h + 1) * 128, :], ot[:])
                else:
                    for half in range(2):
                        nc.scalar.dma_start(
                            agin_h[half][h * 128:(h + 1) * 128, :],
                            ot[:, half * 256:(half + 1) * 256])

            # ---------------- output projection ----------------
            def rhs_load(src, w):
                rhs_tiles = []
                for et in range(ND):
                    rt = yrhsp.tile([128, 512], bf16, tag="yr")
                    nc.sync.dma_start(
                        rt[:, 0:w], src[et * 128:(et + 1) * 128, :])
                    rhs_tiles.append(rt)
                return rhs_tiles

            def project_mms(k, rhs_tiles, i0=0, w=512):
                for ft in range(ESH // 128):
                    py = psB.tile([128, 512], f32, tag="B")
                    for et in range(ND):
                        nc.tensor.matmul(
                            py[:, 0:w], wo_sb[et][:, ft * 128:(ft + 1) * 128],
                            rhs_tiles[et][:, 0:w],
                            start=(et == 0), stop=(et == ND - 1))
                    ys = ysp.tile([128, 512], f32, tag="ys")
                    nc.scalar.activation(ys[:, 0:w], py[:, 0:w],
                                         mybir.ActivationFunctionType.Copy)
                    nc.scalar.dma_start(
                        yT[ft * 128:(ft + 1) * 128,
                           k * 512 + i0:k * 512 + i0 + w],
                        ys[:, 0:w])

            def all_gather(in_ap, out_ap):
                nc.gpsimd.collective_compute(
                    "AllGather",
                    mybir.AluOpType.bypass,
                    replica_groups=[list(range(NCORES))],
                    ins=[in_ap],
                    outs=[out_ap],
                )

            # ---------------- fused pipeline ----------------
            prev_rhs = None
            for ci in range(NCHUNK):
                b, c = divmod(ci, S // 512)
                if ci >= 1:
                    prev_rhs = rhs_load(agout[ci - 1], 512)
                qkv_chunk(ci)
                flush_pending()
                pts0, pacc0 = scores_phase(0, b, c)
                if ci >= 1:
                    project_mms(ci - 1, prev_rhs)
                finish_phase(0, b, c, pts0, pacc0)
                pts1, pacc1 = scores_phase(1, b, c)
                finish_phase(1, b, c, pts1, pacc1)
                if ci < NCHUNK - 1:
                    all_gather(agin[ci].ap().opt(), agout[ci].ap().opt())
                else:
                    # last chunk: two half-width AllGathers so the first
                    # half's projection overlaps the second half's gather
                    for half in range(2):
                        all_gather(agin_h[half].ap().opt(),
                                   agout_h[half].ap().opt())
            for half in range(2):
                rhs = rhs_load(agout_h[half], 256)
                project_mms(NCHUNK - 1, rhs, i0=half * 256, w=256)
    nc.finalize()
    return nc


def _prep_inputs(x, wq, wk, wv, wo, freqs_cos, freqs_sin, mask):
    bf16 = ml_dtypes.bfloat16
    xf = np.ascontiguousarray(x.reshape(T, D).T).astype(bf16)
    # rope tables expanded to head-dim width; sin carries the rotation signs
    cosf = np.repeat(freqs_cos, 2, axis=1).astype(bf16)          # [S, 128]
    sinf = np.repeat(freqs_sin, 2, axis=1)
    sinf = (sinf * np.tile([-1.0, 1.0], HD // 2)[None, :]).astype(bf16)
    # transposed diagonal mask blocks, stacked: [S, 128]
    mdT = np.concatenate(
        [np.ascontiguousarray(mask[i * 128:(i + 1) * 128,
                                   i * 128:(i + 1) * 128].T)
         for i in range(S // 128)], axis=0).astype(np.float32)
    in_maps = []
    for c in range(NCORES):
        sl = slice(c * ESH, (c + 1) * ESH)
        wcatT = np.concatenate(
            [wq[sl, :].T, wk[sl, :].T, wv[sl, :].T], axis=1).astype(bf16)
        woTc = np.ascontiguousarray(wo[sl, :].T).astype(bf16)
        in_maps.append({
            "xT": xf,
            "wcatT": np.ascontiguousarray(wcatT),
            "woT": woTc,
            "cosf": cosf,
            "sinf": sinf,
            "maskdT": mdT,
        })
    return in_maps


def kernel(x, wq, wk, wv, wo, freqs_cos, freqs_sin, mask, start_pos):
    from concourse.bass_utils import run_bass_kernel_spmd

    x = np.asarray(x, dtype=np.float32)
    in_maps = _prep_inputs(
        np.asarray(x, np.float32), np.asarray(wq, np.float32),
        np.asarray(wk, np.float32), np.asarray(wv, np.float32),
        np.asarray(wo, np.float32), np.asarray(freqs_cos, np.float32),
        np.asarray(freqs_sin, np.float32), np.asarray(mask, np.float32))

    if "nc" not in _cache:
        _cache["nc"] = _build_nc()
    res = run_bass_kernel_spmd(_cache["nc"], in_maps, core_ids=list(range(NCORES)))
    _cache["last_result"] = res

    y = np.empty((T, D), dtype=np.float32)
    for c in range(NCORES):
        y[:, c * ESH:(c + 1) * ESH] = np.asarray(res.results[c]["out"]).T
    return y.reshape(B, S, D)
